# revision 2
# baseline (speedup 1.0000x reference)
"""Margin-softmax loss kernel for Trainium2 (8 NeuronCores, SPMD data parallel).

Device (per core, raw Bass, no Tile): stream the [128, 100000] f32 shard of x
through ScalarE exp(S*x) with the fused per-row accumulate; SP issues triple-
buffered 5.12MB DMA loads. Output: stats[128, n_chunks] of per-chunk row sums.
Host: O(B) epilogue — fold chunk sums, gather target logits, log/mean (the
all-reduce of per-device partials).

Sync protocol (walrus here caps embedded sync-waits at 1 per instruction, so
all waits are standalone wait_ge on the consuming engine's sequencer):
  - dma_sem[j] (one per SBUF slot j): each load of slot j adds +16 (one inc
    per SDMA engine). ACT waits dma_sem[j] >= 16*(use+1) before reading use-th
    load of slot j — this requires all 16 engines to have finished that use.
  - act_sem: ACT +1 per chunk. SP waits act_sem >= i-NB+1 before re-loading a
    slot, and >= N_CHUNKS before storing stats.
"""

from contextlib import ExitStack

import numpy as np

S = 64.0
MARGIN = 0.35
B, C = 1024, 100000
N_CORES = 8
P = B // N_CORES  # 128 rows per core = SBUF partitions
W = 10000         # columns per DMA chunk (5.12 MB per dma_start)
N_CHUNKS = C // W
NB = 3            # input slots (triple buffering)

_CACHE = {}


def _build():
    from concourse import bass, mybir

    f32 = mybir.dt.float32
    Exp = mybir.ActivationFunctionType.Exp

    nc = bass.Bass()
    x = nc.dram_tensor("x", [P, C], f32, kind="ExternalInput")
    stats_out = nc.dram_tensor("stats", [P, N_CHUNKS], f32, kind="ExternalOutput")

    with ExitStack() as es:
        slots = [
            es.enter_context(nc.sbuf_tensor(f"t{j}", [P, W], f32)) for j in range(NB)
        ]
        stats = es.enter_context(nc.sbuf_tensor("stats_sb", [P, N_CHUNKS], f32))
        warmb = es.enter_context(nc.sbuf_tensor("warm", [P, 1], f32))
        blk = es.enter_context(nc.Block())
        dma_sems = [
            es.enter_context(nc.semaphore(f"dma_sem{j}")) for j in range(NB)
        ]
        act_sem = es.enter_context(nc.semaphore("act_sem"))

        @blk.sync
        def _(sync):
            for i in range(N_CHUNKS):
                if i >= NB:
                    sync.wait_ge(act_sem, i - NB + 1)
                sync.dma_start(
                    out=slots[i % NB][:, :], in_=x[:, i * W : (i + 1) * W]
                ).then_inc(dma_sems[i % NB], 16)
            sync.wait_ge(act_sem, N_CHUNKS)
            sync.dma_start(out=stats_out[:, :], in_=stats[:, :]).then_inc(
                dma_sems[0], 16
            )

        @blk.scalar
        def _(scalar):
            # First ACTIVATE triggers the exp table-set load (~2.7us) — run it
            # on garbage while chunk 0's DMA is in flight. Output unused.
            scalar.activation(warmb[:, :], warmb[:, :], Exp, scale=1.0)
            for i in range(N_CHUNKS):
                scalar.wait_ge(dma_sems[i % NB], 16 * (i // NB + 1))
                t = slots[i % NB][:, :]
                scalar.activation(
                    t, t, Exp, scale=S, accum_out=stats[:, i : i + 1]
                ).then_inc(act_sem, 1)

    return nc


def _stats_device(x):
    from concourse.bass_utils import run_bass_kernel_spmd

    nc = _CACHE.get("nc")
    if nc is None:
        nc = _build()
        _CACHE["nc"] = nc
    in_maps = [
        {"x": np.ascontiguousarray(x[c * P : (c + 1) * P])} for c in range(N_CORES)
    ]
    res = run_bass_kernel_spmd(
        nc,
        in_maps,
        list(range(N_CORES)),
        trace=_CACHE.get("trace", False),
        tmpdir=_CACHE.get("tmpdir"),
    )
    _CACHE["last"] = res
    return np.stack([res.results[c]["stats"] for c in range(N_CORES)])


def kernel(x, label):
    x = np.asarray(x)
    label = np.asarray(label)

    stats = _stats_device(x)  # [N_CORES, P, N_CHUNKS]
    rowsum = stats.astype(np.float64).sum(axis=2).reshape(B)

    x_y = x[np.arange(B), label.astype(np.int64)].astype(np.float64)
    numerator = S * (x_y - MARGIN)
    sum_excl = rowsum - np.exp(S * x_y)
    denominator = np.exp(numerator) + sum_excl
    L = (numerator - np.log(denominator)) / S
    return np.asarray(-np.mean(L), dtype=np.float32)


# revision 4
# speedup vs baseline: 1.0209x; 1.0209x over previous
"""Margin-softmax loss kernel for Trainium2 (8 NeuronCores, SPMD data parallel).

Device (per core, raw Bass, no Tile): stream the [128, 100000] f32 shard of x
through ScalarE exp(S*x) with the fused per-row accumulate; SP issues triple-
buffered 5.12MB DMA loads. Output: stats[128, n_chunks] of per-chunk row sums.
Host: O(B) epilogue — fold chunk sums, gather target logits, log/mean (the
all-reduce of per-device partials).

Sync protocol (walrus here caps embedded sync-waits at 1 per instruction, so
all waits are standalone wait_ge on the consuming engine's sequencer):
  - dma_sem[j] (one per SBUF slot j): each load of slot j adds +16 (one inc
    per SDMA engine). ACT waits dma_sem[j] >= 16*(use+1) before reading use-th
    load of slot j — this requires all 16 engines to have finished that use.
  - act_sem: ACT +1 per chunk. SP waits act_sem >= i-NB+1 before re-loading a
    slot, and >= N_CHUNKS before storing stats.
"""

from contextlib import ExitStack

import numpy as np

S = 64.0
MARGIN = 0.35
B, C = 1024, 100000
N_CORES = 8
P = B // N_CORES  # 128 rows per core = SBUF partitions
W = 10000         # columns per big DMA chunk (5.12 MB per dma_start)
NB = 3            # big-chunk slots (triple buffering)
# The last-arriving chunk bounds the ScalarE tail after the DMA stream ends,
# so taper the end of the stream down to 500 columns. Taper chunks get
# dedicated slots (no reuse -> no act_sem waits -> the DMA queue never
# stalls near the end).
TAPER = [4000, 2500, 2000, 1000, 500]
N_BIG = (C - sum(TAPER)) // W  # 9
CHUNKS = [W] * N_BIG + TAPER   # column widths, in stream order
N_CHUNKS = len(CHUNKS)
OFFS = [sum(CHUNKS[:i]) for i in range(N_CHUNKS)]

_CACHE = {}


def _build():
    from concourse import bass, mybir

    f32 = mybir.dt.float32
    Exp = mybir.ActivationFunctionType.Exp

    nc = bass.Bass()
    x = nc.dram_tensor("x", [P, C], f32, kind="ExternalInput")
    stats_out = nc.dram_tensor("stats", [P, N_CHUNKS], f32, kind="ExternalOutput")

    with ExitStack() as es:
        big_slots = [
            es.enter_context(nc.sbuf_tensor(f"t{j}", [P, W], f32)) for j in range(NB)
        ]
        taper_slots = [
            es.enter_context(nc.sbuf_tensor(f"tt{k}", [P, w], f32))
            for k, w in enumerate(TAPER)
        ]
        stats = es.enter_context(nc.sbuf_tensor("stats_sb", [P, N_CHUNKS], f32))
        warmb = es.enter_context(nc.sbuf_tensor("warm", [P, 1], f32))
        blk = es.enter_context(nc.Block())
        dma_sems = [
            es.enter_context(nc.semaphore(f"dma_sem{j}")) for j in range(NB)
        ]
        taper_sems = [
            es.enter_context(nc.semaphore(f"taper_sem{k}")) for k in range(len(TAPER))
        ]
        act_sem = es.enter_context(nc.semaphore("act_sem"))

        def slot_sem(i):
            if i < N_BIG:
                return big_slots[i % NB], dma_sems[i % NB], 16 * (i // NB + 1)
            return taper_slots[i - N_BIG], taper_sems[i - N_BIG], 16

        @blk.sync
        def _(sync):
            for i in range(N_CHUNKS):
                slot, sem, _ = slot_sem(i)
                if NB <= i < N_BIG:
                    sync.wait_ge(act_sem, i - NB + 1)
                sync.dma_start(
                    out=slot[:, :], in_=x[:, OFFS[i] : OFFS[i] + CHUNKS[i]]
                ).then_inc(sem, 16)
            sync.wait_ge(act_sem, N_CHUNKS)
            sync.dma_start(out=stats_out[:, :], in_=stats[:, :]).then_inc(
                dma_sems[0], 16
            )

        @blk.scalar
        def _(scalar):
            # First ACTIVATE triggers the exp table-set load (~2.7us) — run it
            # on garbage while chunk 0's DMA is in flight. Output unused.
            scalar.activation(warmb[:, :], warmb[:, :], Exp, scale=1.0)
            for i in range(N_CHUNKS):
                slot, sem, val = slot_sem(i)
                scalar.wait_ge(sem, val)
                t = slot[:, :]
                scalar.activation(
                    t, t, Exp, scale=S, accum_out=stats[:, i : i + 1]
                ).then_inc(act_sem, 1)

    return nc


def _stats_device(x):
    from concourse.bass_utils import run_bass_kernel_spmd

    nc = _CACHE.get("nc")
    if nc is None:
        nc = _build()
        _CACHE["nc"] = nc
    in_maps = [
        {"x": np.ascontiguousarray(x[c * P : (c + 1) * P])} for c in range(N_CORES)
    ]
    res = run_bass_kernel_spmd(
        nc,
        in_maps,
        list(range(N_CORES)),
        trace=_CACHE.get("trace", False),
        tmpdir=_CACHE.get("tmpdir"),
    )
    _CACHE["last"] = res
    return np.stack([res.results[c]["stats"] for c in range(N_CORES)])


def kernel(x, label):
    x = np.asarray(x)
    label = np.asarray(label)

    stats = _stats_device(x)  # [N_CORES, P, N_CHUNKS]
    rowsum = stats.astype(np.float64).sum(axis=2).reshape(B)

    x_y = x[np.arange(B), label.astype(np.int64)].astype(np.float64)
    numerator = S * (x_y - MARGIN)
    sum_excl = rowsum - np.exp(S * x_y)
    denominator = np.exp(numerator) + sum_excl
    L = (numerator - np.log(denominator)) / S
    return np.asarray(-np.mean(L), dtype=np.float32)


# revision 5
# speedup vs baseline: 1.0530x; 1.0314x over previous
"""Margin-softmax loss kernel for Trainium2 (8 NeuronCores, SPMD data parallel).

Device (per core, raw Bass, no Tile): stream the [128, 100000] f32 shard of x
through ScalarE exp(S*x) with the fused per-row accumulate; SP issues triple-
buffered 5.12MB DMA loads. Output: stats[128, n_chunks] of per-chunk row sums.
Host: O(B) epilogue — fold chunk sums, gather target logits, log/mean (the
all-reduce of per-device partials).

Sync protocol (walrus here caps embedded sync-waits at 1 per instruction, so
all waits are standalone wait_ge on the consuming engine's sequencer):
  - dma_sem[j] (one per SBUF slot j): each load of slot j adds +16 (one inc
    per SDMA engine). ACT waits dma_sem[j] >= 16*(use+1) before reading use-th
    load of slot j — this requires all 16 engines to have finished that use.
  - act_sem: ACT +1 per chunk. SP waits act_sem >= i-NB+1 before re-loading a
    slot, and >= N_CHUNKS before storing stats.
"""

from contextlib import ExitStack

import numpy as np

S = 64.0
MARGIN = 0.35
B, C = 1024, 100000
N_CORES = 8
P = B // N_CORES  # 128 rows per core = SBUF partitions
W = 7500          # columns per big DMA chunk (3.84 MB per dma_start)
NB = 3            # big-chunk slots (triple buffering)
# After the last big chunk lands, ScalarE still owes all remaining exp work,
# so the end of the stream decays geometrically per the flatness condition
# dma_time(c_next) = act_time(c) (1.182*c_next = 0.833*c + 400ns): ScalarE
# finishes each chunk just as the next one lands. Taper chunks get dedicated
# slots (no reuse -> no act_sem waits -> the DMA queue never stalls).
TAPER = [5774, 4305, 3373, 2716, 2253, 1927, 1697, 1535, 1420]
N_BIG = (C - sum(TAPER)) // W  # 9
CHUNKS = [W] * N_BIG + TAPER   # column widths, in stream order
N_CHUNKS = len(CHUNKS)
OFFS = [sum(CHUNKS[:i]) for i in range(N_CHUNKS)]

_CACHE = {}


def _build():
    from concourse import bass, mybir

    f32 = mybir.dt.float32
    Exp = mybir.ActivationFunctionType.Exp

    nc = bass.Bass()
    x = nc.dram_tensor("x", [P, C], f32, kind="ExternalInput")
    stats_out = nc.dram_tensor("stats", [P, N_CHUNKS], f32, kind="ExternalOutput")

    with ExitStack() as es:
        big_slots = [
            es.enter_context(nc.sbuf_tensor(f"t{j}", [P, W], f32)) for j in range(NB)
        ]
        taper_slots = [
            es.enter_context(nc.sbuf_tensor(f"tt{k}", [P, w], f32))
            for k, w in enumerate(TAPER)
        ]
        stats = es.enter_context(nc.sbuf_tensor("stats_sb", [P, N_CHUNKS], f32))
        warmb = es.enter_context(nc.sbuf_tensor("warm", [P, 1], f32))
        blk = es.enter_context(nc.Block())
        dma_sems = [
            es.enter_context(nc.semaphore(f"dma_sem{j}")) for j in range(NB)
        ]
        taper_sems = [
            es.enter_context(nc.semaphore(f"taper_sem{k}")) for k in range(len(TAPER))
        ]
        act_sem = es.enter_context(nc.semaphore("act_sem"))

        def slot_sem(i):
            if i < N_BIG:
                return big_slots[i % NB], dma_sems[i % NB], 16 * (i // NB + 1)
            return taper_slots[i - N_BIG], taper_sems[i - N_BIG], 16

        @blk.sync
        def _(sync):
            for i in range(N_CHUNKS):
                slot, sem, _ = slot_sem(i)
                if NB <= i < N_BIG:
                    sync.wait_ge(act_sem, i - NB + 1)
                sync.dma_start(
                    out=slot[:, :], in_=x[:, OFFS[i] : OFFS[i] + CHUNKS[i]]
                ).then_inc(sem, 16)
            sync.wait_ge(act_sem, N_CHUNKS)
            sync.dma_start(out=stats_out[:, :], in_=stats[:, :]).then_inc(
                dma_sems[0], 16
            )

        @blk.scalar
        def _(scalar):
            # First ACTIVATE triggers the exp table-set load (~2.7us) — run it
            # on garbage while chunk 0's DMA is in flight. Output unused.
            scalar.activation(warmb[:, :], warmb[:, :], Exp, scale=1.0)
            for i in range(N_CHUNKS):
                slot, sem, val = slot_sem(i)
                scalar.wait_ge(sem, val)
                t = slot[:, :]
                scalar.activation(
                    t, t, Exp, scale=S, accum_out=stats[:, i : i + 1]
                ).then_inc(act_sem, 1)

    return nc


def _stats_device(x):
    from concourse.bass_utils import run_bass_kernel_spmd

    nc = _CACHE.get("nc")
    if nc is None:
        nc = _build()
        _CACHE["nc"] = nc
    in_maps = [
        {"x": np.ascontiguousarray(x[c * P : (c + 1) * P])} for c in range(N_CORES)
    ]
    res = run_bass_kernel_spmd(
        nc,
        in_maps,
        list(range(N_CORES)),
        trace=_CACHE.get("trace", False),
        tmpdir=_CACHE.get("tmpdir"),
    )
    _CACHE["last"] = res
    return np.stack([res.results[c]["stats"] for c in range(N_CORES)])


def kernel(x, label):
    x = np.asarray(x)
    label = np.asarray(label)

    stats = _stats_device(x)  # [N_CORES, P, N_CHUNKS]
    rowsum = stats.astype(np.float64).sum(axis=2).reshape(B)

    x_y = x[np.arange(B), label.astype(np.int64)].astype(np.float64)
    numerator = S * (x_y - MARGIN)
    sum_excl = rowsum - np.exp(S * x_y)
    denominator = np.exp(numerator) + sum_excl
    L = (numerator - np.log(denominator)) / S
    return np.asarray(-np.mean(L), dtype=np.float32)
